# revision 6
# baseline (speedup 1.0000x reference)
"""EMA (first-order linear recurrence) kernel for Trainium2, 8 NeuronCores.

Problem: y[b, t, d] = a*y[b, t-1, d] + (1-a)*x[b, t, d],  y[b, -1, d] = 0,
x shape (4, 4096, 2048) f32, ALPHA = 0.99.

Strategy (bf16 data path; the correctness gate is rel_err < 2e-2 and bf16
keeps us ~3e-3, while halving HBM traffic and tunnel transfer bytes):
  - Shard (batch, d_model/2) over the 8 cores: core (b, h) handles
    x[b, :, h*1024:(h+1)*1024] of shape (4096, 1024), cast to bf16 on host.
  - Chunked scan over seq: 32 blocks of L=128 steps, grouped by 8 for the
    carry computation.  For each block j:
      w-matmul: stationary one-hot-column matrix (col j = (1-a)*a^(127-k))
        accumulates the block's local end-state w_j into row j of a
        persistent PSUM tile W_psum [32, 1024] (+0 rows elsewhere).
    Per group of 8 blocks (after their w-matmuls):
      ScalarE copies the known prefix W_psum[0:k] -> SBUF (bf16);
      one small matmul computes all carries e_j = sum_{m<j} (aL)^(j-1-m) w_m;
      carries are staged to partition 0 and folded into the INPUT:
        X[row0 of block j] += (a/(1-a)) * e_j
      which makes the plain block-local matmul produce the exact global
      scan: T @ X~_j = T @ X_j + P (x) e_j  (P[i] = a^(i+1)), because only
      T's column 0 touches row 0 and (1-a)*a^i * (a/(1-a))*e = a^(i+1)*e.
    Main matmul per block: Y_j = T @ X~_j -> PSUM, one engine copy (with
      f32->bf16 cast) to an output staging tile, one 1-MiB store DMA per 4
      blocks.  No second fixup pass, no K=1 matmuls, no vector adds.
  - X is resident in SBUF (64 KiB/partition bf16); loads are 8 DMAs of
    1 MiB (4 blocks each, rearranged so partition = t-within-block).

Dispatch: the jitted shard_map executable, the device-resident zero output
buffers, and the Bass module are all cached at module level, so a warm
kernel() call does no re-tracing and ships only x (bf16) through the axon
tunnel and y (bf16) back.

The walrus build in this container supports only ONE sync-wait per
instruction; _legalize_waits splits extras onto same-engine NOPs (see
baseline notes), which is semantics-preserving because engines execute
their streams in order.
"""
import numpy as np
from contextlib import ExitStack

ALPHA = 0.99
B, S, DM = 4, 4096, 2048
NCORES = 8
DS = 1024                  # d-columns per core (d_model/2)
L = 128                    # seq block length
NB = S // L                # 32 blocks
GRP = 8                    # blocks per carry-scan group
NH = 512                   # matmul moving-operand half (PSUM f32 bank limit)
LCH = 8                    # blocks per load DMA (2 MiB bf16), sync/SP ring
SCH = 8                    # blocks per store DMA (2 MiB bf16), scalar/ACT ring

_cache = {}


# ---------------------------------------------------------------------------
# walrus wait-count legalization
# ---------------------------------------------------------------------------
def _legalize_waits(nc, max_waits=1, matmul_max=0):
    import concourse.mybir as mybir

    counter = [0]

    def split(blk):
        insts = blk.instructions
        i = 0
        while i < len(insts):
            inst = insts[i]
            for sub in (getattr(inst, "blocks", None) or []):
                split(sub)
            si = inst.sync_info
            cap = matmul_max if isinstance(inst, mybir.InstMatmult) else max_waits
            if si is not None and si.on_wait and len(si.on_wait) > cap:
                waits = list(si.on_wait)
                keep = waits[len(waits) - cap:] if cap > 0 else []
                overflow = waits[: len(waits) - cap]
                nops = []
                for j in range(0, len(overflow), max_waits):
                    chunk = overflow[j: j + max_waits]
                    counter[0] += 1
                    nop = mybir.InstNoOp(name=f"wsplit_nop_{counter[0]}")
                    nop.engine = inst.engine
                    nop.sync_info = mybir.SyncInfo(on_wait=chunk, on_update=[])
                    nops.append(nop)
                inst.sync_info = mybir.SyncInfo(
                    on_wait=keep, on_update=list(si.on_update)
                )
                for k, nop in enumerate(nops):
                    insts.insert(i + k, nop)
                i += len(nops)
            i += 1

    for fn in nc.m.functions:
        for blk in fn.blocks:
            split(blk)
    return nc


# ---------------------------------------------------------------------------
# constants
# ---------------------------------------------------------------------------
def _np_bf16():
    import ml_dtypes

    return np.dtype(ml_dtypes.bfloat16)


def _constants():
    a = float(ALPHA)
    bf16 = _np_bf16()
    ii = np.arange(L)
    diff = ii[None, :] - ii[:, None]              # i - k
    # tT[k, i] = T[i, k] = (1-a) * a^(i-k) for k <= i else 0
    tT = np.where(
        diff >= 0,
        (1.0 - a) * np.power(a, np.clip(diff, 0, None).astype(np.float64)),
        0.0,
    ).astype(bf16)
    # wsel[:, j*NB:(j+1)*NB] is the stationary [L, NB] for block j's w-matmul:
    # only column j is nonzero, = tlast[k] = (1-a) * a^(L-1-k).
    tlast = ((1.0 - a) * np.power(a, (L - 1 - ii).astype(np.float64)))
    wsel = np.zeros((L, NB * NB), dtype=np.float64)
    for j in range(NB):
        wsel[:, j * NB + j] = tlast
    wsel = wsel.astype(bf16)
    # sT[m, j] = (aL)^(j-1-m) for m <= j-1 else 0  (e_j = sum_m sT[m,j] w_m)
    jj = np.arange(NB)
    djj = jj[None, :] - 1 - jj[:, None]
    aL = a ** L
    sT = np.where(
        djj >= 0, np.power(aL, np.clip(djj, 0, None).astype(np.float64)), 0.0
    ).astype(bf16)
    return tT, wsel, sT


def _build_nc(reps=1):
    import concourse.bass as bass
    import concourse.tile as tile
    from concourse import mybir

    f32 = mybir.dt.float32
    bf16 = mybir.dt.bfloat16
    AL = mybir.AluOpType
    tT_np, wsel_np, sT_np = _constants()

    nc = bass.Bass("TRN2", target_bir_lowering=False, debug=False)
    x = nc.dram_tensor("x_sh", [S, DS], bf16, kind="ExternalInput")
    y = nc.dram_tensor("y_sh", [S, DS], bf16, kind="ExternalOutput")
    tT_d = nc.inline_tensor(tT_np, name="tT_const")
    wsel_d = nc.inline_tensor(wsel_np, name="wsel_const")
    sT_d = nc.inline_tensor(sT_np, name="sT_const")

    with ExitStack() as ctx:
        tc = ctx.enter_context(tile.TileContext(nc))
        cpool = ctx.enter_context(tc.tile_pool(name="cpool", bufs=1))
        opool = ctx.enter_context(tc.tile_pool(name="opool", bufs=3))
        cspool = ctx.enter_context(tc.tile_pool(name="cspool", bufs=2))
        ppt = ctx.enter_context(tc.tile_pool(name="ppt", bufs=2, space="PSUM"))
        pw = ctx.enter_context(tc.tile_pool(name="pw", bufs=1, space="PSUM"))
        pct = ctx.enter_context(tc.tile_pool(name="pct", bufs=1, space="PSUM"))

        tT = cpool.tile([L, L], bf16)
        nc.sync.dma_start(tT[:], tT_d.ap())
        wsel = cpool.tile([L, NB * NB], bf16)
        nc.sync.dma_start(wsel[:], wsel_d.ap())
        sT = cpool.tile([NB, NB], bf16)
        nc.sync.dma_start(sT[:], sT_d.ap())

        X = cpool.tile([L, NB * DS], bf16, tag="X")     # resident input
        Ws = cpool.tile([NB, DS], bf16, tag="Ws")       # block end states
        C = cpool.tile([NB, DS], bf16, tag="C")         # inter-block carries
        W_psum = pw.tile([NB, DS], f32, tag="Wp")       # persistent w rows

        xap, yap = x.ap(), y.ap()

        rep_loop = tc.For_i(0, reps, 1) if reps > 1 else None
        if rep_loop is not None:
            rep_loop.__enter__()
        if True:
            # ---------------- loads: 8 x 1 MiB ----------------
            for q in range(NB // LCH):
                dst = X[:, q * LCH * DS:(q + 1) * LCH * DS].rearrange(
                    "p (b n) -> p b n", b=LCH
                )
                src = xap[q * LCH * L:(q + 1) * LCH * L, :].rearrange(
                    "(b p) n -> p b n", b=LCH
                )
                nc.sync.dma_start(dst, src)

            for g in range(NB // GRP):
                j0 = g * GRP
                k = j0 + GRP
                # ---------- w-matmuls for this group ----------
                for j in range(j0, k):
                    for h in range(DS // NH):
                        nc.tensor.matmul(
                            W_psum[:, h * NH:(h + 1) * NH],
                            wsel[:, j * NB:(j + 1) * NB],
                            X[:, j * DS + h * NH: j * DS + (h + 1) * NH],
                            start=(j == 0), stop=(j == NB - 1),
                            skip_group_check=True,
                        )
                # ---------- carry chain ----------
                nc.scalar.copy(Ws[0:k, :], W_psum[0:k, :])
                ct = pct.tile([NB, DS], f32, tag="ct", name=f"ct{g}")
                for h in range(DS // NH):
                    nc.tensor.matmul(
                        ct[0:k, h * NH:(h + 1) * NH],
                        sT[0:k, 0:k],
                        Ws[0:k, h * NH:(h + 1) * NH],
                        start=True, stop=True,
                    )
                nc.scalar.copy(C[0:k, :], ct[0:k, :])
                cs = cspool.tile([1, GRP * DS], bf16, tag="cs", name=f"cs{g}")
                nc.scalar.dma_start(
                    cs[0:1, :].rearrange("p (j n) -> p j n", j=GRP),
                    C[j0:k, :],
                )
                # fold carries into row 0 of each block of X
                nc.vector.scalar_tensor_tensor(
                    X[0:1, j0 * DS:k * DS],
                    cs[0:1, :],
                    float(ALPHA / (1.0 - ALPHA)),
                    X[0:1, j0 * DS:k * DS],
                    op0=AL.mult,
                    op1=AL.add,
                )
                # ---------- main matmuls + copies + stores ----------
                for j in range(j0, k):
                    pt = ppt.tile([L, DS], f32, tag="pt", name=f"pt{j}")
                    for h in range(DS // NH):
                        nc.tensor.matmul(
                            pt[:, h * NH:(h + 1) * NH],
                            tT[:],
                            X[:, j * DS + h * NH: j * DS + (h + 1) * NH],
                            start=True, stop=True,
                        )
                    if j % SCH == 0:
                        ot = opool.tile(
                            [L, SCH * DS], bf16, tag="ot", name=f"ot{j // SCH}"
                        )
                    dst_sl = ot[:, (j % SCH) * DS:((j % SCH) + 1) * DS]
                    if j % 2 == 0:
                        nc.scalar.copy(dst_sl, pt[:])
                    else:
                        nc.vector.tensor_copy(dst_sl, pt[:])
                    if j % SCH == SCH - 1:
                        jb = j - (SCH - 1)
                        nc.scalar.dma_start(
                            yap[jb * L:(j + 1) * L, :].rearrange(
                                "(b p) n -> p b n", b=SCH
                            ),
                            ot[:].rearrange("p (b n) -> p b n", b=SCH),
                        )
        if rep_loop is not None:
            rep_loop.__exit__(None, None, None)
    return _legalize_waits(nc)


def _get_nc():
    if "nc" not in _cache:
        _cache["nc"] = _build_nc()
    return _cache["nc"]


# ---------------------------------------------------------------------------
# cached jitted dispatch (mirrors bass2jax.run_bass_via_pjrt, but the traced
# executable, mesh, and zero output buffers are built once and reused)
# ---------------------------------------------------------------------------
def _get_exec():
    if "exec" in _cache:
        return _cache["exec"]
    import jax
    from jax.sharding import Mesh, PartitionSpec, NamedSharding
    from jax.experimental.shard_map import shard_map
    import concourse.mybir as mybir
    from concourse import bass2jax

    bass2jax.install_neuronx_cc_hook()
    nc = _get_nc()

    partition_name = nc.partition_id_tensor.name if nc.partition_id_tensor else None
    in_names, out_names, out_avals = [], [], []
    for alloc in nc.m.functions[0].allocations:
        if not isinstance(alloc, mybir.MemoryLocationSet):
            continue
        name = alloc.memorylocations[0].name
        if alloc.kind == "ExternalInput":
            if name != partition_name:
                in_names.append(name)
        elif alloc.kind == "ExternalOutput":
            out_names.append(name)
            out_avals.append(
                jax.core.ShapedArray(
                    tuple(alloc.tensor_shape), mybir.dt.np(alloc.dtype)
                )
            )
    all_names = list(in_names) + list(out_names)
    if partition_name is not None:
        all_names.append(partition_name)

    def _body(*args):
        operands = list(args)
        if partition_name is not None:
            operands.append(bass2jax.partition_id_tensor())
        return tuple(
            bass2jax._bass_exec_p.bind(
                *operands,
                out_avals=tuple(out_avals),
                in_names=tuple(all_names),
                out_names=tuple(out_names),
                lowering_input_output_aliases=(),
                sim_require_finite=True,
                sim_require_nnan=True,
                nc=nc,
            )
        )

    devices = jax.devices()[:NCORES]
    mesh = Mesh(np.asarray(devices), ("core",))
    nin = len(in_names) + len(out_names)
    sharded = jax.jit(
        shard_map(
            _body, mesh=mesh,
            in_specs=(PartitionSpec("core"),) * nin,
            out_specs=(PartitionSpec("core"),) * len(out_names),
            check_rep=False,
        ),
        keep_unused=True,
    )
    sharding = NamedSharding(mesh, PartitionSpec("core"))
    # device-resident zero buffers for the ExternalOutput operands; NOT
    # donated, so they are created once and reused every call.
    zeros = [
        jax.device_put(
            np.zeros((NCORES * a.shape[0], *a.shape[1:]), a.dtype), sharding
        )
        for a in out_avals
    ]
    _cache["exec"] = (sharded, sharding, devices, zeros)
    return _cache["exec"]


def _shard_cast(x):
    """Per-core bf16 slices: core c = (b, h) -> x[b, :, h*DS:(h+1)*DS]."""
    bf16 = _np_bf16()
    return [
        x[c // 2, :, (c % 2) * DS:((c % 2) + 1) * DS].astype(bf16)
        for c in range(NCORES)
    ]


def kernel(x) -> np.ndarray:
    import jax

    x = np.asarray(x, dtype=np.float32)
    assert x.shape == (B, S, DM), x.shape
    sharded, sharding, devices, zeros = _get_exec()
    parts = _shard_cast(x)
    dparts = [jax.device_put(parts[c], devices[c]) for c in range(NCORES)]
    xg = jax.make_array_from_single_device_arrays(
        (NCORES * S, DS), sharding, dparts
    )
    outs = sharded(xg, *zeros)
    g = np.asarray(outs[0]).reshape(B, 2, S, DS)
    out = np.empty((B, S, DM), dtype=np.float32)
    out[:, :, :DS] = g[:, 0]
    out[:, :, DS:] = g[:, 1]
    return out
